# revision 12
# baseline (speedup 1.0000x reference)
"""DLRM-small forward on 8 Trainium2 NeuronCores (Bass/Tile).

Strategy: data-parallel over the batch (2048 samples/core), embedding table
replicated in each core's DRAM as bf16. Per core:
  - bot MLP 13->512->256->128 in f32r (full-rate fp32 matmul, ~tf32 precision)
  - embedding rows gathered with indirect DMA (bf16, 128 rows/instruction),
    transposed to [d=128, lookups] with the HWDGE xbar DMA transpose, and
    assembled into a per-sample "stackT" layout (bf16, 32-aligned slots)
  - pairwise interactions as per-4-sample PE matmuls (bf16 -> fp32 PSUM)
  - xact redistributed to a K-dense [128, batch] layout (XBAND) with
    cross-quadrant DVE stream_shuffles, rounded to f32r
  - top MLP layer 1 consumes XBAND against a host-prebuilt symmetrized
    [7*128, 1024] weight (f32r); layers 2..5 standard K-tiled f32r matmuls
Output: [16384, 1] fp32.
"""
import sys
import types
import numpy as np
import ml_dtypes

bf16 = ml_dtypes.bfloat16

# ---- problem constants (hardcoded; kernel.py must be self-contained) ----
NUM_TABLES = 26
VOCAB_PER_TABLE = 100000
TOTAL_VOCAB = NUM_TABLES * VOCAB_PER_TABLE
EMBED_DIM = 128
NUM_DENSE = 13
BOT_DIMS = [512, 256, 128]
TOP_DIMS = [1024, 1024, 512, 256, 1]
BATCH = 16384
N_CORES = 8
B_CORE = BATCH // N_CORES      # 2048
BT = 512                       # batch tile (N of the MLP matmuls)
CH = 64                        # stage-I / ST chunk (samples per psum batch)
NSLOT = 27                     # bot_out + 26 embeddings
GCOL = 128                     # lookup rows per gather instruction
LPC = CH * NUM_TABLES          # lookups per chunk (1664 = 13*128)

OFFSETS = (np.arange(NUM_TABLES) * VOCAB_PER_TABLE).astype(np.int32)
TRIU_I, TRIU_J = np.triu_indices(1 + NUM_TABLES)

_CACHE = {}
TRACE = False


def _install_profile_hook():
    try:
        from antenv.axon_hooks import get_axon_ntff_profile_hook  # noqa: F401
        return
    except ImportError:
        pass
    try:
        import antenv
        mod = types.ModuleType("antenv.axon_hooks")
        _state = {"hook": None}
        mod.set_axon_ntff_profile_hook = lambda h: _state.__setitem__("hook", h)
        mod.get_axon_ntff_profile_hook = lambda: _state["hook"]
        sys.modules["antenv.axon_hooks"] = mod
        antenv.axon_hooks = mod
        from trn_agent_boot.trn_boot import _ntff_profile_via_ctypes
        mod.set_axon_ntff_profile_hook(_ntff_profile_via_ctypes("/opt/axon/libaxon_pjrt.so"))
    except Exception:
        pass


def _st_copy_segments(g):
    """ET block g covers chunk lookups L in [128g, 128g+128), L = 26*b + t.
    Return (src_off, dst_col_off, nb, width) segments: head/body/tail."""
    segs = []  # (src_off, b0, slot0, nb, width)
    L0 = 128 * g
    b0, t0 = divmod(L0, 26)
    src = 0
    if t0 > 0:
        c0 = 26 - t0
        segs.append((src, b0, t0 + 1, 1, c0))
        src += c0
        b0 += 1
    nb = (128 - src) // 26
    if nb > 0:
        segs.append((src, b0, 1, nb, 26))
        src += nb * 26
        b0 += nb
    rem = 128 - src
    if rem > 0:
        segs.append((src, b0, 1, 1, rem))
    return segs


def build_nc(b_core=B_CORE, n_cores=N_CORES):
    import concourse.bass as bass
    import concourse.mybir as mybir
    import concourse.tile as tile
    from concourse import bacc

    f32 = mybir.dt.float32
    f32r = mybir.dt.float32r
    b16 = mybir.dt.bfloat16
    i32 = mybir.dt.int32

    nbt = b_core // BT                  # 4
    nch = BT // CH                      # 8
    ncol_ch = LPC // GCOL               # 13
    ncol = (b_core // CH) * ncol_ch     # 416

    nc = bacc.Bacc("TRN2", target_bir_lowering=False, debug=False,
                   num_devices=n_cores)

    # ---- DRAM I/O ----
    emb_d = nc.dram_tensor("emb", [TOTAL_VOCAB, EMBED_DIM], b16, kind="ExternalInput").ap()
    denseT_d = nc.dram_tensor("denseT", [NUM_DENSE, b_core], f32r, kind="ExternalInput").ap()
    idx_d = nc.dram_tensor("idx", [GCOL, ncol], i32, kind="ExternalInput").ap()
    bw_d = [nc.dram_tensor(f"bw{i}", [k, m], f32r, kind="ExternalInput").ap()
            for i, (k, m) in enumerate([(13, 512), (512, 256), (256, 128)])]
    bb_d = [nc.dram_tensor(f"bb{i}", [128, m // 128], f32, kind="ExternalInput").ap()
            for i, m in enumerate(BOT_DIMS)]
    w0a_d = nc.dram_tensor("w0a", [128, 1024], f32r, kind="ExternalInput").ap()
    w0f_d = nc.dram_tensor("w0f", [7 * 128, 1024], f32r, kind="ExternalInput").ap()
    tw_d = [nc.dram_tensor(f"tw{i}", [k, m], f32r, kind="ExternalInput").ap()
            for i, (k, m) in enumerate([(1024, 1024), (1024, 512), (512, 256), (256, 1)], start=1)]
    tb_d = [nc.dram_tensor("tb0", [128, 8], f32, kind="ExternalInput").ap(),
            nc.dram_tensor("tb1", [128, 8], f32, kind="ExternalInput").ap(),
            nc.dram_tensor("tb2", [128, 4], f32, kind="ExternalInput").ap(),
            nc.dram_tensor("tb3", [128, 2], f32, kind="ExternalInput").ap(),
            nc.dram_tensor("tb4", [1, 1], f32, kind="ExternalInput").ap()]
    out_d = nc.dram_tensor("out", [1, b_core], f32, kind="ExternalOutput").ap()

    Relu = mybir.ActivationFunctionType.Relu
    Ident = mybir.ActivationFunctionType.Identity
    IDMASK = list(range(32))

    with tile.TileContext(nc) as tc:
        with (
            tc.tile_pool(name="wsb", bufs=1) as wsb,
            tc.tile_pool(name="io", bufs=1) as io,
            tc.tile_pool(name="est", bufs=24) as est,
            tc.tile_pool(name="etp", bufs=8) as etp,
            tc.tile_pool(name="stp", bufs=3) as stp,
            tc.tile_pool(name="xbp", bufs=1) as xbp,
            tc.tile_pool(name="hp", bufs=1) as hp,
            tc.tile_pool(name="botp", bufs=4) as botp,
            tc.tile_pool(name="osb", bufs=2) as osb,
            tc.tile_pool(name="ps_x", bufs=1, space="PSUM") as ps_x,
            tc.tile_pool(name="ps_t", bufs=1, space="PSUM") as ps_t,
            tc.tile_pool(name="ps_y", bufs=1, space="PSUM") as ps_y,
            tc.tile_pool(name="ps_m", bufs=2, space="PSUM") as ps_m,
        ):
            # ---- load constants / weights ----
            from concourse.masks import make_identity
            ident = wsb.tile([128, 128], b16, tag="ident")
            make_identity(nc, ident[:])
            denseT = io.tile([NUM_DENSE, b_core], f32r, tag="denseT")
            nc.sync.dma_start(out=denseT[:], in_=denseT_d[:])
            idx_sb = io.tile([GCOL, ncol], i32, tag="idx")
            nc.sync.dma_start(out=idx_sb[:], in_=idx_d[:])

            bw = []
            for i, (k, m) in enumerate([(13, 512), (512, 256), (256, 128)]):
                t = wsb.tile([k if k <= 128 else 128, (max(k // 128, 1)) * m], f32r,
                             tag=f"bw{i}", name=f"bw{i}_sb")
                if k <= 128:
                    nc.sync.dma_start(out=t[:], in_=bw_d[i][:])
                else:
                    for kt in range(k // 128):
                        nc.sync.dma_start(out=t[:, kt * m:(kt + 1) * m],
                                          in_=bw_d[i][kt * 128:(kt + 1) * 128, :])
                bw.append(t)
            bb = []
            for i, m in enumerate(BOT_DIMS):
                t = wsb.tile([128, m // 128], f32, tag=f"bb{i}", name=f"bb{i}_sb")
                nc.sync.dma_start(out=t[:], in_=bb_d[i][:])
                bb.append(t)

            w0a = wsb.tile([128, 1024], f32r, tag="w0a")
            nc.sync.dma_start(out=w0a[:], in_=w0a_d[:])
            w0f = wsb.tile([128, 7 * 1024], f32r, tag="w0f")
            for g in range(7):
                nc.sync.dma_start(out=w0f[:, g * 1024:(g + 1) * 1024],
                                  in_=w0f_d[g * 128:(g + 1) * 128, :])
            tw = []
            for i, (k, m) in enumerate([(1024, 1024), (1024, 512), (512, 256), (256, 1)], start=1):
                t = wsb.tile([128, (k // 128) * m], f32r, tag=f"tw{i}", name=f"tw{i}_sb")
                for kt in range(k // 128):
                    nc.sync.dma_start(out=t[:, kt * m:(kt + 1) * m],
                                      in_=tw_d[i - 1][kt * 128:(kt + 1) * 128, :])
                tw.append(t)
            tb = []
            for i, m in enumerate([8, 8, 4, 2]):
                t = wsb.tile([128, m], f32, tag=f"tb{i}", name=f"tb{i}_sb")
                nc.sync.dma_start(out=t[:], in_=tb_d[i][:])
                tb.append(t)
            tb4 = wsb.tile([1, 1], f32, tag="tb4")
            nc.sync.dma_start(out=tb4[:], in_=tb_d[4][:])

            # ---- bot MLP for all batch tiles up front (PE warm-up work) ----
            bot_outs = []
            for bt in range(nbt):
                bs = bt * BT
                hb1 = [hp.tile([128, BT], f32r, tag=f"hb1_{mc}", name=f"hb1_{mc}") for mc in range(4)]
                for mc in range(4):
                    pm = ps_m.tile([128, BT], f32, tag="mlp")
                    nc.tensor.matmul(pm[:], bw[0][:, mc * 128:(mc + 1) * 128],
                                     denseT[:, bs:bs + BT], start=True, stop=True)
                    nc.scalar.activation(hb1[mc][:], pm[:], Relu, bias=bb[0][:, mc:mc + 1])
                hb2 = [hp.tile([128, BT], f32r, tag=f"hb2_{mc}", name=f"hb2_{mc}") for mc in range(2)]
                for mc in range(2):
                    pm = ps_m.tile([128, BT], f32, tag="mlp")
                    for kt in range(4):
                        nc.tensor.matmul(pm[:], bw[1][:, kt * 256 + mc * 128: kt * 256 + (mc + 1) * 128],
                                         hb1[kt][:], start=(kt == 0), stop=(kt == 3))
                    nc.scalar.activation(hb2[mc][:], pm[:], Relu, bias=bb[1][:, mc:mc + 1])
                bot_outT = botp.tile([128, BT], f32r, tag="bot_outT", name=f"bot_outT_{bt}")
                pm = ps_m.tile([128, BT], f32, tag="mlp")
                for kt in range(2):
                    nc.tensor.matmul(pm[:], bw[2][:, kt * 128:(kt + 1) * 128],
                                     hb2[kt][:], start=(kt == 0), stop=(kt == 1))
                nc.scalar.activation(bot_outT[:], pm[:], Relu, bias=bb[2][:, 0:1])
                bot_outs.append(bot_outT)

            # ---- per batch-tile: interactions + top MLP ----
            for bt in range(nbt):
                bs = bt * BT
                bot_outT = bot_outs[bt]
                xband = xbp.tile([128, 7 * BT], f32, tag="xband")
                xband_r = xbp.tile([128, 7 * BT], f32r, tag="xband_r")

                for ch in range(nch):
                    st = stp.tile([128, 32 * CH], b16, tag="st")
                    nc.vector.memset(st[:].rearrange("p (b s) -> p b s", s=32)[:, :, 27:32], 0)
                    nc.vector.tensor_copy(
                        out=st[:].rearrange("p (b s) -> p b s", s=32)[:, :, 0:1],
                        in_=bot_outT[:, ch * CH:(ch + 1) * CH].rearrange("p (b o) -> p b o", o=1),
                    )
                    for g in range(ncol_ch):
                        col = (bt * nch + ch) * ncol_ch + g
                        e_t = est.tile([GCOL, 128], b16, tag="e")
                        nc.gpsimd.indirect_dma_start(
                            out=e_t[:],
                            out_offset=None,
                            in_=emb_d[:],
                            in_offset=bass.IndirectOffsetOnAxis(ap=idx_sb[:, col:col + 1], axis=0),
                        )
                        pt = ps_t.tile([128, GCOL], b16, tag="pt")
                        nc.tensor.transpose(out=pt[:], in_=e_t[:], identity=ident[:])
                        et = pt
                        st3 = st[:].rearrange("p (b s) -> p b s", s=32)
                        for (src_off, sb0, slot0, nb, width) in _st_copy_segments(g):
                            if nb > 1:
                                nc.scalar.copy(
                                    out=st3[:, sb0:sb0 + nb, slot0:slot0 + width],
                                    in_=et[:, src_off:src_off + nb * width].rearrange("p (b s) -> p b s", s=width),
                                )
                            else:
                                dst_off = 32 * sb0 + slot0
                                nc.scalar.copy(
                                    out=st[:, dst_off:dst_off + width],
                                    in_=et[:, src_off:src_off + width],
                                )
                    # stage-I interactions
                    px = ps_x.tile([128, 16 * 128], f32, tag="px")
                    for g4 in range(CH // 4):
                        nc.tensor.matmul(px[:, g4 * 128:(g4 + 1) * 128],
                                         st[:, g4 * 128:(g4 + 1) * 128],
                                         st[:, g4 * 128:(g4 + 1) * 128],
                                         start=True, stop=True)
                    # band shuffles into xband (fp32)
                    for a in range(4):
                        src = px[:].rearrange("p (q f) -> p q f", f=128)[32 * a:32 * a + 32, :, 32 * a + 0:32 * a + 28]
                        src = src.rearrange("p q (g i) -> p q g i", i=4)
                        for i in range(4):
                            dst = xband[32 * i:32 * i + 32, :].rearrange("p (g b) -> p g b", b=BT)
                            dst = dst[:, :, ch * CH:(ch + 1) * CH].rearrange("p g (q a) -> p g q a", a=4)
                            nc.vector.stream_shuffle(
                                out=dst[:, :, :, a].rearrange("p g q -> p q g"),
                                in_=src[:, :, :, i],
                                mask=IDMASK,
                            )
                nc.vector.tensor_copy(out=xband_r[:], in_=xband[:])

                # top layer 1
                h1 = [hp.tile([128, BT], f32r, tag=f"hA_{jc}", name=f"h1_{jc}") for jc in range(8)]
                for jc in range(8):
                    py = ps_y.tile([128, BT], f32, tag="py")
                    nc.tensor.matmul(py[:], w0a[:, jc * 128:(jc + 1) * 128],
                                     bot_outT[:], start=True, stop=False)
                    for g in range(7):
                        nc.tensor.matmul(py[:], w0f[:, g * 1024 + jc * 128: g * 1024 + (jc + 1) * 128],
                                         xband_r[:, g * BT:(g + 1) * BT],
                                         start=False, stop=(g == 6))
                    nc.scalar.activation(h1[jc][:], py[:], Relu, bias=tb[0][:, jc:jc + 1])
                # layer 2: 1024 -> 1024
                h2 = [hp.tile([128, BT], f32r, tag=f"hB_{jc}", name=f"h2_{jc}") for jc in range(8)]
                for jc in range(8):
                    pm = ps_m.tile([128, BT], f32, tag="mlp")
                    for kt in range(8):
                        nc.tensor.matmul(pm[:], tw[0][:, kt * 1024 + jc * 128: kt * 1024 + (jc + 1) * 128],
                                         h1[kt][:], start=(kt == 0), stop=(kt == 7))
                    nc.scalar.activation(h2[jc][:], pm[:], Relu, bias=tb[1][:, jc:jc + 1])
                # layer 3: 1024 -> 512 (reuses hA slots)
                h3 = [hp.tile([128, BT], f32r, tag=f"hA_{jc}", name=f"h3_{jc}") for jc in range(4)]
                for jc in range(4):
                    pm = ps_m.tile([128, BT], f32, tag="mlp")
                    for kt in range(8):
                        nc.tensor.matmul(pm[:], tw[1][:, kt * 512 + jc * 128: kt * 512 + (jc + 1) * 128],
                                         h2[kt][:], start=(kt == 0), stop=(kt == 7))
                    nc.scalar.activation(h3[jc][:], pm[:], Relu, bias=tb[2][:, jc:jc + 1])
                # layer 4: 512 -> 256 (reuses hB slots)
                h4 = [hp.tile([128, BT], f32r, tag=f"hB_{jc}", name=f"h4_{jc}") for jc in range(2)]
                for jc in range(2):
                    pm = ps_m.tile([128, BT], f32, tag="mlp")
                    for kt in range(4):
                        nc.tensor.matmul(pm[:], tw[2][:, kt * 256 + jc * 128: kt * 256 + (jc + 1) * 128],
                                         h3[kt][:], start=(kt == 0), stop=(kt == 3))
                    nc.scalar.activation(h4[jc][:], pm[:], Relu, bias=tb[3][:, jc:jc + 1])
                # layer 5: 256 -> 1
                pm = ps_m.tile([1, BT], f32, tag="mlp")
                for kt in range(2):
                    nc.tensor.matmul(pm[:], tw[3][:, kt * 1:(kt + 1) * 1],
                                     h4[kt][:], start=(kt == 0), stop=(kt == 1))
                o_sb = osb.tile([1, BT], f32, tag="o")
                nc.scalar.activation(o_sb[:], pm[:], Ident, bias=tb4[:, 0:1])
                nc.sync.dma_start(out=out_d[:, bs:bs + BT], in_=o_sb[:])

    nc.finalize()
    return nc


def prep_weights(inputs):
    """Host-side weight/bias reshaping shared by all cores (fp32 arrays)."""
    w = {}
    for i in range(3):
        w[f"bw{i}"] = np.ascontiguousarray(inputs[f"bot_w{i}"], np.float32)
        m = BOT_DIMS[i]
        w[f"bb{i}"] = np.ascontiguousarray(
            inputs[f"bot_b{i}"].reshape(m // 128, 128).T, np.float32)
    top_w0 = np.asarray(inputs["top_w0"], np.float32)       # [506, 1024]
    w["w0a"] = np.ascontiguousarray(top_w0[:128, :])
    pos = np.zeros((NSLOT, NSLOT), np.int64)
    pos[TRIU_I, TRIU_J] = np.arange(len(TRIU_I))
    pos[TRIU_J, TRIU_I] = np.arange(len(TRIU_I))
    wsym = top_w0[128 + pos.reshape(-1), :].reshape(NSLOT, NSLOT, 1024)
    off = (~np.eye(NSLOT, dtype=bool))[:, :, None]
    wsym = wsym * np.where(off, 0.5, 1.0).astype(np.float32)
    w0f = np.zeros((7, 128, 1024), np.float32)
    for g in range(7):
        for i in range(4):
            n = 4 * g + i
            if n < NSLOT:
                w0f[g, 32 * i:32 * i + 27, :] = wsym[n]
    w["w0f"] = w0f.reshape(7 * 128, 1024)
    for i in range(1, 5):
        w[f"tw{i}"] = np.ascontiguousarray(inputs[f"top_w{i}"], np.float32)
    for i, m in enumerate([1024, 1024, 512, 256]):
        w[f"tb{i}"] = np.ascontiguousarray(
            inputs[f"top_b{i}"].reshape(m // 128, 128).T, np.float32)
    w["tb4"] = np.asarray(inputs["top_b4"], np.float32).reshape(1, 1)
    return w


def make_idx(cat_core):
    """[b_core, 26] int32 -> [128, ncol] gather layout (chunk-major cols)."""
    b_core = cat_core.shape[0]
    flat = (cat_core.astype(np.int64) + OFFSETS[None, :].astype(np.int64)).astype(np.int32)
    flat = flat.reshape(b_core // CH, LPC)                   # [nchunk, 1664]
    ncol_ch = LPC // GCOL
    idx = np.empty((GCOL, flat.shape[0] * ncol_ch), np.int32)
    for c in range(flat.shape[0]):
        blk = flat[c].reshape(ncol_ch, GCOL).T               # [128, 13]
        idx[:, c * ncol_ch:(c + 1) * ncol_ch] = blk
    return idx


def kernel(**inputs) -> np.ndarray:
    _install_profile_hook()
    from concourse import bass_utils

    dense = np.asarray(inputs["dense"], np.float32)          # [16384, 13]
    cat = np.asarray(inputs["cat"], np.int32)                # [16384, 26]
    emb = np.asarray(inputs["emb_table"], np.float32)        # [2.6M, 128]

    if "nc" not in _CACHE:
        _CACHE["nc"] = build_nc()
    nc = _CACHE["nc"]

    w = prep_weights(inputs)
    emb_bf = emb.astype(bf16)

    in_maps = []
    for c in range(N_CORES):
        s = slice(c * B_CORE, (c + 1) * B_CORE)
        m = dict(w)
        m["emb"] = emb_bf
        m["denseT"] = np.ascontiguousarray(dense[s].T)
        m["idx"] = make_idx(cat[s])
        in_maps.append(m)

    res = bass_utils.run_bass_kernel_spmd(
        nc, in_maps, core_ids=list(range(N_CORES)), trace=TRACE,
    )
    out = np.concatenate([res.results[c]["out"].reshape(B_CORE, 1)
                          for c in range(N_CORES)], axis=0)
    kernel.last_result = res
    return out.astype(np.float32)


# revision 13
# speedup vs baseline: 1.2637x; 1.2637x over previous
"""DLRM-small forward on 8 Trainium2 NeuronCores (Bass/Tile).

Strategy: data-parallel over the batch (2048 samples/core), embedding table
replicated in each core's DRAM as bf16. Per core:
  - bot MLP 13->512->256->128 in f32r (full-rate fp32 matmul, ~tf32 precision)
  - embedding rows gathered with indirect DMA (bf16, 128 rows/instruction),
    transposed to [d=128, lookups] with the HWDGE xbar DMA transpose, and
    assembled into a per-sample "stackT" layout (bf16, 32-aligned slots)
  - pairwise interactions as per-4-sample PE matmuls (bf16 -> fp32 PSUM)
  - xact redistributed to a K-dense [128, batch] layout (XBAND) with
    cross-quadrant DVE stream_shuffles, rounded to f32r
  - top MLP layer 1 consumes XBAND against a host-prebuilt symmetrized
    [7*128, 1024] weight (f32r); layers 2..5 standard K-tiled f32r matmuls
Output: [16384, 1] fp32.
"""
import sys
import types
import numpy as np
import ml_dtypes

bf16 = ml_dtypes.bfloat16

# ---- problem constants (hardcoded; kernel.py must be self-contained) ----
NUM_TABLES = 26
VOCAB_PER_TABLE = 100000
TOTAL_VOCAB = NUM_TABLES * VOCAB_PER_TABLE
EMBED_DIM = 128
NUM_DENSE = 13
BOT_DIMS = [512, 256, 128]
TOP_DIMS = [1024, 1024, 512, 256, 1]
BATCH = 16384
N_CORES = 8
B_CORE = BATCH // N_CORES      # 2048
BT = 512                       # batch tile (N of the MLP matmuls)
CH = 64                        # stage-I / ST chunk (samples per psum batch)
NSLOT = 27                     # bot_out + 26 embeddings
GCOL = 128                     # lookup rows per gather instruction
LPC = CH * NUM_TABLES          # lookups per chunk (1664 = 13*128)

OFFSETS = (np.arange(NUM_TABLES) * VOCAB_PER_TABLE).astype(np.int32)
TRIU_I, TRIU_J = np.triu_indices(1 + NUM_TABLES)

_CACHE = {}
TRACE = False


def _install_profile_hook():
    try:
        from antenv.axon_hooks import get_axon_ntff_profile_hook  # noqa: F401
        return
    except ImportError:
        pass
    try:
        import antenv
        mod = types.ModuleType("antenv.axon_hooks")
        _state = {"hook": None}
        mod.set_axon_ntff_profile_hook = lambda h: _state.__setitem__("hook", h)
        mod.get_axon_ntff_profile_hook = lambda: _state["hook"]
        sys.modules["antenv.axon_hooks"] = mod
        antenv.axon_hooks = mod
        from trn_agent_boot.trn_boot import _ntff_profile_via_ctypes
        mod.set_axon_ntff_profile_hook(_ntff_profile_via_ctypes("/opt/axon/libaxon_pjrt.so"))
    except Exception:
        pass


def _st_copy_segments(g):
    """ET block g covers chunk lookups L in [128g, 128g+128), L = 26*b + t.
    Return (src_off, dst_col_off, nb, width) segments: head/body/tail."""
    segs = []  # (src_off, b0, slot0, nb, width)
    L0 = 128 * g
    b0, t0 = divmod(L0, 26)
    src = 0
    if t0 > 0:
        c0 = 26 - t0
        segs.append((src, b0, t0 + 1, 1, c0))
        src += c0
        b0 += 1
    nb = (128 - src) // 26
    if nb > 0:
        segs.append((src, b0, 1, nb, 26))
        src += nb * 26
        b0 += nb
    rem = 128 - src
    if rem > 0:
        segs.append((src, b0, 1, 1, rem))
    return segs


def build_nc(b_core=B_CORE, n_cores=N_CORES):
    import concourse.bass as bass
    import concourse.mybir as mybir
    import concourse.tile as tile
    from concourse import bacc

    f32 = mybir.dt.float32
    f32r = mybir.dt.float32r
    b16 = mybir.dt.bfloat16
    i32 = mybir.dt.int32

    nbt = b_core // BT                  # 4
    nch = BT // CH                      # 8
    ncol_ch = LPC // GCOL               # 13
    ncol = (b_core // CH) * ncol_ch     # 416

    nc = bacc.Bacc("TRN2", target_bir_lowering=False, debug=False,
                   num_devices=n_cores)

    # ---- DRAM I/O ----
    emb_d = nc.dram_tensor("emb", [TOTAL_VOCAB, EMBED_DIM], b16, kind="ExternalInput").ap()
    denseT_d = nc.dram_tensor("denseT", [NUM_DENSE, b_core], f32r, kind="ExternalInput").ap()
    idx_d = nc.dram_tensor("idx", [GCOL, ncol], i32, kind="ExternalInput").ap()
    bw_d = [nc.dram_tensor(f"bw{i}", [k, m], f32r, kind="ExternalInput").ap()
            for i, (k, m) in enumerate([(13, 512), (512, 256), (256, 128)])]
    bb_d = [nc.dram_tensor(f"bb{i}", [128, m // 128], f32, kind="ExternalInput").ap()
            for i, m in enumerate(BOT_DIMS)]
    w0a_d = nc.dram_tensor("w0a", [128, 1024], f32r, kind="ExternalInput").ap()
    w0f_d = nc.dram_tensor("w0f", [7 * 128, 1024], f32r, kind="ExternalInput").ap()
    tw_d = [nc.dram_tensor(f"tw{i}", [k, m], f32r, kind="ExternalInput").ap()
            for i, (k, m) in enumerate([(1024, 1024), (1024, 512), (512, 256), (256, 1)], start=1)]
    tb_d = [nc.dram_tensor("tb0", [128, 8], f32, kind="ExternalInput").ap(),
            nc.dram_tensor("tb1", [128, 8], f32, kind="ExternalInput").ap(),
            nc.dram_tensor("tb2", [128, 4], f32, kind="ExternalInput").ap(),
            nc.dram_tensor("tb3", [128, 2], f32, kind="ExternalInput").ap(),
            nc.dram_tensor("tb4", [1, 1], f32, kind="ExternalInput").ap()]
    out_d = nc.dram_tensor("out", [1, b_core], f32, kind="ExternalOutput").ap()

    Relu = mybir.ActivationFunctionType.Relu
    Ident = mybir.ActivationFunctionType.Identity
    IDMASK = list(range(32))

    with tile.TileContext(nc) as tc:
        with (
            tc.tile_pool(name="wsb", bufs=1) as wsb,
            tc.tile_pool(name="io", bufs=1) as io,
            tc.tile_pool(name="est", bufs=24) as est,
            tc.tile_pool(name="etp", bufs=8) as etp,
            tc.tile_pool(name="stp", bufs=4) as stp,
            tc.tile_pool(name="xbp", bufs=1) as xbp,
            tc.tile_pool(name="hp", bufs=1) as hp,
            tc.tile_pool(name="botp", bufs=4) as botp,
            tc.tile_pool(name="osb", bufs=2) as osb,
            tc.tile_pool(name="ps_x", bufs=1, space="PSUM") as ps_x,
            tc.tile_pool(name="ps_t", bufs=2, space="PSUM") as ps_t,
            tc.tile_pool(name="ps_m", bufs=2, space="PSUM") as ps_m,
        ):
            # ---- load constants / weights ----
            from concourse.masks import make_identity
            ident = wsb.tile([128, 128], b16, tag="ident")
            make_identity(nc, ident[:])
            denseT = io.tile([NUM_DENSE, b_core], f32r, tag="denseT")
            nc.sync.dma_start(out=denseT[:], in_=denseT_d[:])
            idx_sb = io.tile([GCOL, ncol], i32, tag="idx")
            nc.sync.dma_start(out=idx_sb[:], in_=idx_d[:])

            bw = []
            for i, (k, m) in enumerate([(13, 512), (512, 256), (256, 128)]):
                t = wsb.tile([k if k <= 128 else 128, (max(k // 128, 1)) * m], f32r,
                             tag=f"bw{i}", name=f"bw{i}_sb")
                if k <= 128:
                    nc.sync.dma_start(out=t[:], in_=bw_d[i][:])
                else:
                    for kt in range(k // 128):
                        nc.sync.dma_start(out=t[:, kt * m:(kt + 1) * m],
                                          in_=bw_d[i][kt * 128:(kt + 1) * 128, :])
                bw.append(t)
            bb = []
            for i, m in enumerate(BOT_DIMS):
                t = wsb.tile([128, m // 128], f32, tag=f"bb{i}", name=f"bb{i}_sb")
                nc.sync.dma_start(out=t[:], in_=bb_d[i][:])
                bb.append(t)

            w0a = wsb.tile([128, 1024], f32r, tag="w0a")
            nc.sync.dma_start(out=w0a[:], in_=w0a_d[:])
            w0f = wsb.tile([128, 7 * 1024], f32r, tag="w0f")
            for g in range(7):
                nc.sync.dma_start(out=w0f[:, g * 1024:(g + 1) * 1024],
                                  in_=w0f_d[g * 128:(g + 1) * 128, :])
            tw = []
            for i, (k, m) in enumerate([(1024, 1024), (1024, 512), (512, 256), (256, 1)], start=1):
                t = wsb.tile([128, (k // 128) * m], f32r, tag=f"tw{i}", name=f"tw{i}_sb")
                for kt in range(k // 128):
                    nc.sync.dma_start(out=t[:, kt * m:(kt + 1) * m],
                                      in_=tw_d[i - 1][kt * 128:(kt + 1) * 128, :])
                tw.append(t)
            tb = []
            for i, m in enumerate([8, 8, 4, 2]):
                t = wsb.tile([128, m], f32, tag=f"tb{i}", name=f"tb{i}_sb")
                nc.sync.dma_start(out=t[:], in_=tb_d[i][:])
                tb.append(t)
            tb4 = wsb.tile([1, 1], f32, tag="tb4")
            nc.sync.dma_start(out=tb4[:], in_=tb_d[4][:])

            # ---- bot MLP for all batch tiles up front (PE warm-up work) ----
            bot_outs = []
            for bt in range(nbt):
                bs = bt * BT
                hb1 = [hp.tile([128, BT], f32r, tag=f"hb1_{mc}", name=f"hb1_{mc}") for mc in range(4)]
                for mc in range(4):
                    pm = ps_m.tile([128, BT], f32, tag="mlp")
                    nc.tensor.matmul(pm[:], bw[0][:, mc * 128:(mc + 1) * 128],
                                     denseT[:, bs:bs + BT], start=True, stop=True)
                    nc.scalar.activation(hb1[mc][:], pm[:], Relu, bias=bb[0][:, mc:mc + 1])
                hb2 = [hp.tile([128, BT], f32r, tag=f"hb2_{mc}", name=f"hb2_{mc}") for mc in range(2)]
                for mc in range(2):
                    pm = ps_m.tile([128, BT], f32, tag="mlp")
                    for kt in range(4):
                        nc.tensor.matmul(pm[:], bw[1][:, kt * 256 + mc * 128: kt * 256 + (mc + 1) * 128],
                                         hb1[kt][:], start=(kt == 0), stop=(kt == 3))
                    nc.scalar.activation(hb2[mc][:], pm[:], Relu, bias=bb[1][:, mc:mc + 1])
                bot_outT = botp.tile([128, BT], f32r, tag="bot_outT", name=f"bot_outT_{bt}")
                pm = ps_m.tile([128, BT], f32, tag="mlp")
                for kt in range(2):
                    nc.tensor.matmul(pm[:], bw[2][:, kt * 128:(kt + 1) * 128],
                                     hb2[kt][:], start=(kt == 0), stop=(kt == 1))
                nc.scalar.activation(bot_outT[:], pm[:], Relu, bias=bb[2][:, 0:1])
                bot_outs.append(bot_outT)

            # ---- per batch-tile: interactions + top MLP ----
            for bt in range(nbt):
                bs = bt * BT
                bot_outT = bot_outs[bt]
                xband = xbp.tile([128, 7 * BT], f32, tag="xband")
                xband_r = xbp.tile([128, 7 * BT], f32r, tag="xband_r")

                for ch in range(nch):
                    st = stp.tile([128, 32 * CH], b16, tag="st")
                    nc.vector.memset(st[:].rearrange("p (b s) -> p b s", s=32)[:, :, 27:32], 0)
                    nc.vector.tensor_copy(
                        out=st[:].rearrange("p (b s) -> p b s", s=32)[:, :, 0:1],
                        in_=bot_outT[:, ch * CH:(ch + 1) * CH].rearrange("p (b o) -> p b o", o=1),
                    )
                    for g in range(ncol_ch):
                        col = (bt * nch + ch) * ncol_ch + g
                        e_t = est.tile([GCOL, 128], b16, tag="e")
                        nc.gpsimd.indirect_dma_start(
                            out=e_t[:],
                            out_offset=None,
                            in_=emb_d[:],
                            in_offset=bass.IndirectOffsetOnAxis(ap=idx_sb[:, col:col + 1], axis=0),
                        )
                        pt = ps_t.tile([128, GCOL], b16, tag="pt")
                        nc.tensor.transpose(out=pt[:], in_=e_t[:], identity=ident[:])
                        et = pt
                        st3 = st[:].rearrange("p (b s) -> p b s", s=32)
                        ceng = nc.scalar if (g % 2 == 0) else nc.vector
                        for (src_off, sb0, slot0, nb, width) in _st_copy_segments(g):
                            if nb > 1:
                                _src = et[:, src_off:src_off + nb * width].rearrange("p (b s) -> p b s", s=width)
                                _dst = st3[:, sb0:sb0 + nb, slot0:slot0 + width]
                            else:
                                dst_off = 32 * sb0 + slot0
                                _src = et[:, src_off:src_off + width]
                                _dst = st[:, dst_off:dst_off + width]
                            if ceng is nc.scalar:
                                nc.scalar.copy(out=_dst, in_=_src)
                            else:
                                nc.vector.tensor_copy(out=_dst, in_=_src)
                    # stage-I interactions
                    px = ps_x.tile([128, 16 * 128], f32, tag="px")
                    for g4 in range(CH // 4):
                        nc.tensor.matmul(px[:, g4 * 128:(g4 + 1) * 128],
                                         st[:, g4 * 128:(g4 + 1) * 128],
                                         st[:, g4 * 128:(g4 + 1) * 128],
                                         start=True, stop=True)
                    # band shuffles into xband (fp32)
                    for a in range(4):
                        src = px[:].rearrange("p (q f) -> p q f", f=128)[32 * a:32 * a + 32, :, 32 * a + 0:32 * a + 28]
                        src = src.rearrange("p q (g i) -> p q g i", i=4)
                        for i in range(4):
                            dst = xband[32 * i:32 * i + 32, :].rearrange("p (b g) -> p b g", g=7)
                            dst = dst[:, ch * CH:(ch + 1) * CH, :].rearrange("p (q a) g -> p q a g", a=4)
                            nc.vector.stream_shuffle(
                                out=dst[:, :, a, :],
                                in_=src[:, :, :, i],
                                mask=IDMASK,
                            )
                nc.vector.tensor_copy(out=xband_r[:], in_=xband[:])

                # top layer 1
                h1 = [hp.tile([128, BT], f32r, tag=f"hA_{jc}", name=f"h1_{jc}") for jc in range(8)]
                for jc in range(8):
                    py = ps_m.tile([128, BT], f32, tag="mlp", name="py")
                    nc.tensor.matmul(py[:], w0a[:, jc * 128:(jc + 1) * 128],
                                     bot_outT[:], start=True, stop=False)
                    for g in range(7):
                        nc.tensor.matmul(py[:], w0f[:, g * 1024 + jc * 128: g * 1024 + (jc + 1) * 128],
                                         xband_r[:].rearrange("p (b g) -> p b g", g=7)[:, :, g],
                                         start=False, stop=(g == 6))
                    nc.scalar.activation(h1[jc][:], py[:], Relu, bias=tb[0][:, jc:jc + 1])
                # layer 2: 1024 -> 1024
                h2 = [hp.tile([128, BT], f32r, tag=f"hB_{jc}", name=f"h2_{jc}") for jc in range(8)]
                for jc in range(8):
                    pm = ps_m.tile([128, BT], f32, tag="mlp")
                    for kt in range(8):
                        nc.tensor.matmul(pm[:], tw[0][:, kt * 1024 + jc * 128: kt * 1024 + (jc + 1) * 128],
                                         h1[kt][:], start=(kt == 0), stop=(kt == 7))
                    nc.scalar.activation(h2[jc][:], pm[:], Relu, bias=tb[1][:, jc:jc + 1])
                # layer 3: 1024 -> 512 (reuses hA slots)
                h3 = [hp.tile([128, BT], f32r, tag=f"hA_{jc}", name=f"h3_{jc}") for jc in range(4)]
                for jc in range(4):
                    pm = ps_m.tile([128, BT], f32, tag="mlp")
                    for kt in range(8):
                        nc.tensor.matmul(pm[:], tw[1][:, kt * 512 + jc * 128: kt * 512 + (jc + 1) * 128],
                                         h2[kt][:], start=(kt == 0), stop=(kt == 7))
                    nc.scalar.activation(h3[jc][:], pm[:], Relu, bias=tb[2][:, jc:jc + 1])
                # layer 4: 512 -> 256 (reuses hB slots)
                h4 = [hp.tile([128, BT], f32r, tag=f"hB_{jc}", name=f"h4_{jc}") for jc in range(2)]
                for jc in range(2):
                    pm = ps_m.tile([128, BT], f32, tag="mlp")
                    for kt in range(4):
                        nc.tensor.matmul(pm[:], tw[2][:, kt * 256 + jc * 128: kt * 256 + (jc + 1) * 128],
                                         h3[kt][:], start=(kt == 0), stop=(kt == 3))
                    nc.scalar.activation(h4[jc][:], pm[:], Relu, bias=tb[3][:, jc:jc + 1])
                # layer 5: 256 -> 1
                pm = ps_m.tile([1, BT], f32, tag="mlp")
                for kt in range(2):
                    nc.tensor.matmul(pm[:], tw[3][:, kt * 1:(kt + 1) * 1],
                                     h4[kt][:], start=(kt == 0), stop=(kt == 1))
                o_sb = osb.tile([1, BT], f32, tag="o")
                nc.scalar.activation(o_sb[:], pm[:], Ident, bias=tb4[:, 0:1])
                nc.sync.dma_start(out=out_d[:, bs:bs + BT], in_=o_sb[:])

    nc.finalize()
    return nc


def prep_weights(inputs):
    """Host-side weight/bias reshaping shared by all cores (fp32 arrays)."""
    w = {}
    for i in range(3):
        w[f"bw{i}"] = np.ascontiguousarray(inputs[f"bot_w{i}"], np.float32)
        m = BOT_DIMS[i]
        w[f"bb{i}"] = np.ascontiguousarray(
            inputs[f"bot_b{i}"].reshape(m // 128, 128).T, np.float32)
    top_w0 = np.asarray(inputs["top_w0"], np.float32)       # [506, 1024]
    w["w0a"] = np.ascontiguousarray(top_w0[:128, :])
    pos = np.zeros((NSLOT, NSLOT), np.int64)
    pos[TRIU_I, TRIU_J] = np.arange(len(TRIU_I))
    pos[TRIU_J, TRIU_I] = np.arange(len(TRIU_I))
    wsym = top_w0[128 + pos.reshape(-1), :].reshape(NSLOT, NSLOT, 1024)
    off = (~np.eye(NSLOT, dtype=bool))[:, :, None]
    wsym = wsym * np.where(off, 0.5, 1.0).astype(np.float32)
    w0f = np.zeros((7, 128, 1024), np.float32)
    for g in range(7):
        for i in range(4):
            n = 4 * g + i
            if n < NSLOT:
                w0f[g, 32 * i:32 * i + 27, :] = wsym[n]
    w["w0f"] = w0f.reshape(7 * 128, 1024)
    for i in range(1, 5):
        w[f"tw{i}"] = np.ascontiguousarray(inputs[f"top_w{i}"], np.float32)
    for i, m in enumerate([1024, 1024, 512, 256]):
        w[f"tb{i}"] = np.ascontiguousarray(
            inputs[f"top_b{i}"].reshape(m // 128, 128).T, np.float32)
    w["tb4"] = np.asarray(inputs["top_b4"], np.float32).reshape(1, 1)
    return w


def make_idx(cat_core):
    """[b_core, 26] int32 -> [128, ncol] gather layout (chunk-major cols)."""
    b_core = cat_core.shape[0]
    flat = (cat_core.astype(np.int64) + OFFSETS[None, :].astype(np.int64)).astype(np.int32)
    flat = flat.reshape(b_core // CH, LPC)                   # [nchunk, 1664]
    ncol_ch = LPC // GCOL
    idx = np.empty((GCOL, flat.shape[0] * ncol_ch), np.int32)
    for c in range(flat.shape[0]):
        blk = flat[c].reshape(ncol_ch, GCOL).T               # [128, 13]
        idx[:, c * ncol_ch:(c + 1) * ncol_ch] = blk
    return idx


def kernel(**inputs) -> np.ndarray:
    _install_profile_hook()
    from concourse import bass_utils

    dense = np.asarray(inputs["dense"], np.float32)          # [16384, 13]
    cat = np.asarray(inputs["cat"], np.int32)                # [16384, 26]
    emb = np.asarray(inputs["emb_table"], np.float32)        # [2.6M, 128]

    if "nc" not in _CACHE:
        _CACHE["nc"] = build_nc()
    nc = _CACHE["nc"]

    w = prep_weights(inputs)
    emb_bf = emb.astype(bf16)

    in_maps = []
    for c in range(N_CORES):
        s = slice(c * B_CORE, (c + 1) * B_CORE)
        m = dict(w)
        m["emb"] = emb_bf
        m["denseT"] = np.ascontiguousarray(dense[s].T)
        m["idx"] = make_idx(cat[s])
        in_maps.append(m)

    res = bass_utils.run_bass_kernel_spmd(
        nc, in_maps, core_ids=list(range(N_CORES)), trace=TRACE,
    )
    out = np.concatenate([res.results[c]["out"].reshape(B_CORE, 1)
                          for c in range(N_CORES)], axis=0)
    kernel.last_result = res
    return out.astype(np.float32)
